# revision 7
# baseline (speedup 1.0000x reference)
"""Trainium2 Bass kernel for nn_CrossTransformer_36756330119370.

The reference module's attention runs over a single key/value position
(k/v are projections of y reshaped to [B*T, 1, C]), so entmax15 over an
axis of length 1 is identically 1.0 and the q/k projections cancel out
of the forward entirely. The computation reduces exactly (verified
bit-identical on CPU) to:

    w[b, t, :] = Wo @ (Wv @ y[b, :, t] + bv) + bo          # [C] per (b,t)
    z[b, c, t, v] = x[b, c, t, v] + w[b, t, c]

Sharding: data-parallel over B across the 8 NeuronCores (8 batches per
core), projection weights replicated. Per core: two small chained fp32
matmuls on the PE engine produce w for the core's 960 (b,t) columns;
then the 24.6MB x-shard is streamed HBM->SBUF, w is added broadcast
over the V axis with a stride-0 access pattern on the vector engine,
and the result streamed back. The kernel is HBM-bandwidth-bound.

All stage-A operands (pre-transposed weights, biases, gathered y) are
packed host-side into one [128, 2948] tensor loaded by a single DMA so
the first PE matmul needs only one sync wait (walrus rejects LDWEIGHTS
instructions with many distinct semaphore waits).
"""

import os
import sys

for _p in ("/opt/trn_rl_repo", "/root/.axon_site/_ro/trn_rl_repo"):
    if os.path.isdir(_p) and _p not in sys.path:
        sys.path.append(_p)

import numpy as np

import concourse.bass as bass
import concourse.mybir as mybir
import concourse.tile as tile
from concourse.bass_utils import run_bass_kernel_spmd

N_CORES = 8
B, C, T, V = 64, 256, 120, 25
BPC = B // N_CORES          # batches per core
P = 128                     # SBUF partitions
NCC = C // P                # channel chunks (2)
BT = BPC * T                # (b, t) columns per core (960)
NT = 480                    # matmul moving-operand tile (<=512 for fp32)
TV = T * V                  # contiguous elements per (b, c) row (3000)

# column offsets inside the packed constant tensor
OFF_WVT = 0                 # [kc, m] -> kc*C + m          (512 cols)
OFF_WOT = NCC * C           # 512, same layout             (512 cols)
OFF_BV = 2 * NCC * C        # 1024: [mc]                   (2 cols)
OFF_BO = OFF_BV + NCC       # 1026                         (2 cols)
OFF_Y = OFF_BO + NCC        # 1028: [kc, b, t] -> kc*BT + b*T + t (1920 cols)
PACK_COLS = OFF_Y + NCC * BT  # 2948

FP32 = mybir.dt.float32

# Stash of the last hardware run results (exec_time_ns etc.) for test.py.
LAST_RESULTS = None


def legalize_waits(nc: bass.Bass, max_waits: int = 1) -> None:
    """Split multi-semaphore waits into standalone NoOp wait carriers.

    The walrus build here rejects any instruction carrying more than one
    sync-wait command ("Too many sync wait commands"), including Tile's
    own kernel-tail Drain. A NoOp on the same engine stalls the
    sequencer identically, so hoisting all but one wait onto NoOps
    preserves semantics.
    """
    k = 0
    for blk in nc.m.functions[0].blocks:
        insts = blk.instructions
        i = 0
        while i < len(insts):
            inst = insts[i]
            si = getattr(inst, "sync_info", None)
            if si is not None and si.on_wait and len(si.on_wait) > max_waits:
                waits = list(si.on_wait)
                for w in waits[:-max_waits]:
                    nop = mybir.InstNoOp(name=f"NW-{k}")
                    k += 1
                    nop.engine = inst.engine
                    nop.sync_info = mybir.SyncInfo(on_wait=[w], on_update=[])
                    insts.insert(i, nop)
                    i += 1
                inst.sync_info = mybir.SyncInfo(
                    on_wait=waits[-max_waits:], on_update=si.on_update)
            i += 1


def build_nc(legalize: bool = True) -> bass.Bass:
    nc = bass.Bass("TRN2", debug=False, num_devices=N_CORES)

    x = nc.dram_tensor("x", [BPC, C, T, V], FP32, kind="ExternalInput").ap()
    cpak = nc.dram_tensor("cpak", [P, PACK_COLS], FP32, kind="ExternalInput").ap()
    z = nc.dram_tensor("z", [BPC, C, T, V], FP32, kind="ExternalOutput").ap()

    with tile.TileContext(nc) as tc:
        with (
            tc.tile_pool(name="const", bufs=1) as cpool,
            tc.tile_pool(name="small", bufs=1) as spool,
            tc.tile_pool(name="psum", bufs=4, space="PSUM") as ppool,
            tc.tile_pool(name="stream", bufs=6) as xpool,
        ):
            # ---- Stage A: w = WoT.T @ (WvT.T @ y + bv) + bo ----
            cs = cpool.tile([P, PACK_COLS], FP32)
            nc.sync.dma_start(cs[:], cpak)

            v_sb = spool.tile([P, NCC, BT], FP32)
            w_sb = spool.tile([P, NCC, BT], FP32)

            def rhs1(kc, nch):
                return cs[:, OFF_Y + kc * BT + nch * NT:
                          OFF_Y + kc * BT + (nch + 1) * NT]

            def rhs2(kc, nch):
                return v_sb[:, kc, nch * NT:(nch + 1) * NT]

            for w_off, b_off, rhs, dst in (
                (OFF_WVT, OFF_BV, rhs1, v_sb),
                (OFF_WOT, OFF_BO, rhs2, w_sb),
            ):
                for mc in range(NCC):
                    for nch in range(BT // NT):
                        pt = ppool.tile([P, NT], FP32, tag="ps")
                        for kc in range(NCC):
                            col = w_off + kc * C + mc * P
                            nc.tensor.matmul(
                                pt[:],
                                lhsT=cs[:, col:col + P],
                                rhs=rhs(kc, nch),
                                start=(kc == 0),
                                stop=(kc == NCC - 1),
                            )
                        # PSUM -> SBUF with per-partition bias add
                        nc.scalar.add(
                            dst[:, mc, nch * NT:(nch + 1) * NT],
                            pt[:],
                            cs[:, b_off + mc:b_off + mc + 1],
                        )

            # ---- Stage B: stream x, add w broadcast over V ----
            for b in range(BPC):
                xt = xpool.tile([P, NCC, TV], FP32)
                nc.sync.dma_start(
                    xt[:], x[b].rearrange("(cc p) t v -> p cc (t v)", p=P)
                )
                xt_v = xt[:].rearrange("p cc (t v) -> p cc t v", v=V)
                w_bc = (
                    w_sb[:, :, b * T:(b + 1) * T]
                    .unsqueeze(3)
                    .broadcast_to([P, NCC, T, V])
                )
                nc.vector.tensor_tensor(xt_v, xt_v, w_bc, mybir.AluOpType.add)
                nc.sync.dma_start(
                    z[b].rearrange("(cc p) t v -> p cc (t v)", p=P), xt[:]
                )

    if legalize:
        # CoreSim can't execute raw-injected NoOps; only legalize for HW.
        legalize_waits(nc)
    return nc


def pack_consts(y_shard, Wv, bv, Wo, bo):
    """Build the [P, PACK_COLS] stage-A constant tensor for one core."""
    cpak = np.empty((P, PACK_COLS), np.float32)
    # wvt[c_in, c_out] = Wv[c_out, c_in]; wvt_sb[p, kc*C + m] = wvt[kc*P+p, m]
    cpak[:, OFF_WVT:OFF_WVT + NCC * C] = (
        Wv.T.reshape(NCC, P, C).transpose(1, 0, 2).reshape(P, NCC * C))
    cpak[:, OFF_WOT:OFF_WOT + NCC * C] = (
        Wo.T.reshape(NCC, P, C).transpose(1, 0, 2).reshape(P, NCC * C))
    cpak[:, OFF_BV:OFF_BV + NCC] = bv.reshape(NCC, P).T
    cpak[:, OFF_BO:OFF_BO + NCC] = bo.reshape(NCC, P).T
    # y_sb[p, kc*BT + b*T + t] = y[b, kc*P+p, t]
    cpak[:, OFF_Y:] = (
        y_shard.reshape(BPC, NCC, P, T).transpose(2, 1, 0, 3).reshape(P, NCC * BT))
    return cpak


_NC_CACHE = None


def _get_nc():
    global _NC_CACHE
    if _NC_CACHE is None:
        _NC_CACHE = build_nc()
    return _NC_CACHE


def kernel(x, y, Wq=None, bq=None, Wk=None, bk=None, Wv=None, bv=None,
           Wo=None, bo=None, **_unused):
    global LAST_RESULTS
    x = np.ascontiguousarray(np.asarray(x, dtype=np.float32))
    y = np.asarray(y, dtype=np.float32)
    Wv = np.asarray(Wv, dtype=np.float32)
    bv = np.asarray(bv, dtype=np.float32)
    Wo = np.asarray(Wo, dtype=np.float32)
    bo = np.asarray(bo, dtype=np.float32)

    nc = _get_nc()
    in_maps = []
    for c in range(N_CORES):
        sl = slice(c * BPC, (c + 1) * BPC)
        in_maps.append({
            "x": x[sl],
            "cpak": pack_consts(y[sl], Wv, bv, Wo, bo),
        })

    res = run_bass_kernel_spmd(
        nc, in_maps, list(range(N_CORES)),
        trace=bool(os.environ.get("KERNEL_PROFILE")),
    )
    LAST_RESULTS = res
    return np.concatenate([res.results[c]["z"] for c in range(N_CORES)], axis=0)
